# revision 1
# baseline (speedup 1.0000x reference)
"""DeformableConv2D Trainium2 Bass kernel.

Problem: x[4,64,64,256] f32, w_offset[3,3,256,27], b_offset[27], filt[256,256,3,3]
  -> out[4,64,64,256] f32  (3x3 deformable conv, DG=1, SAME padding)

Sharding: 8 cores = (batch b = core//2) x (image-row half = core%2).
Each core computes 32 output rows (2048 pixels) of its batch element.

Key layout trick: the host stages a zero-padded, *paired-row* bf16 copy of
the image: record r=(y,x) holds channels of pixels (y,x) AND (y+1,x).  One
2KB dma_gather descriptor per (tap, pixel) then fetches all 4 bilinear
corners [v00|v10|v01|v11] at once, pixel-major.

Per-core device pipeline:
  P3  offset conv (PE, bf16): wi_cm [27, 2048]
  PT  PE-transpose -> pixel-major wi_pm [128px, 27-per-tile]
  P4  DVE/ACT: clamp, floor, bilinear corner weights (mask-folded),
      int16 gather indices (pixel-major => per-partition scalars)
  P5  SWDGE dma_gather: per (pxgroup, tap) fetch [128px, 4tl, 1024] bf16
  P6  scale by w_corner (DVE/ACT tensor_scalar, per-partition) then PE
      matmuls lhsT=scaledG, rhs=I128 accumulating 4 corners in PSUM
      = fused per-pixel-scale + transpose + corner-sum -> sampled [c, px]
  P7  PE matmuls lhsT=sampled[c,px], rhs=W[c,f] -> out [128px, 256f]
"""

import os
import sys
import numpy as np
import ml_dtypes

sys.path.insert(0, "/opt/trn_rl_repo")

BF16 = ml_dtypes.bfloat16

B, H, W, C, F, K, KK = 4, 64, 64, 256, 256, 3, 9
PAD = 6
Wp = 76
SLAB_ROWS = 45           # 44 addressable + 1 zero guard row
SLAB_PX = SLAB_ROWS * Wp  # 3420
NREC = 44 * Wp            # 3344 addressable paired-row records
NPX = 2048
CLAMP = 4.99

_CACHE = {}
LAST_RESULT = None
DEBUG = bool(int(os.environ.get("KERNEL_DEBUG", "0")))


def _build_nc():
    import concourse.bass as bass
    from concourse import bacc, mybir
    import concourse.tile as tile

    dt = mybir.dt
    Alu = mybir.AluOpType
    Act = mybir.ActivationFunctionType

    nc = bacc.Bacc("TRN2", target_bir_lowering=False)

    xrp_d = nc.dram_tensor("xrp", [NREC + 1, 512], dt.bfloat16, kind="ExternalInput")
    xcm_d = nc.dram_tensor("xslab_cm", [128, 2 * SLAB_PX], dt.bfloat16, kind="ExternalInput")
    wmain_d = nc.dram_tensor("wmain", [128, 18 * 256], dt.bfloat16, kind="ExternalInput")
    woff_d = nc.dram_tensor("woff", [128, 2 * 9 * 27], dt.bfloat16, kind="ExternalInput")
    bias_d = nc.dram_tensor("bias", [27, 1], dt.float32, kind="ExternalInput")
    out_d = nc.dram_tensor("out", [NPX, C], dt.float32, kind="ExternalOutput")
    if DEBUG:
        dbg_wicm = nc.dram_tensor("dbg_wicm", [27, NPX], dt.float32, kind="ExternalOutput")
        dbg_wipm = nc.dram_tensor("dbg_wipm", [128, 432], dt.float32, kind="ExternalOutput")
        dbg_w = nc.dram_tensor("dbg_w", [128, 5 * 144], dt.float32, kind="ExternalOutput")
        dbg_idx = nc.dram_tensor("dbg_idx", [128, 144], dt.int16, kind="ExternalOutput")
        dbg_idxw = nc.dram_tensor("dbg_idxw", [128, 1152], dt.int16, kind="ExternalOutput")
        dbg_samp = nc.dram_tensor("dbg_samp", [128, 18 * 512], dt.bfloat16, kind="ExternalOutput")
        dbg_g = nc.dram_tensor("dbg_g", [128, 9 * 4096], dt.bfloat16, kind="ExternalOutput")

    # --- inline constants ---
    base_np = np.zeros((128, 144), dtype=np.float32)
    r = np.arange(128)
    for t in range(16):
        for kk in range(KK):
            ki, kj = kk // 3, kk % 3
            base_np[:, t * 9 + kk] = (2 * t + r // 64 + ki + 5) * Wp + (r % 64) + kj + 5
    base_d = nc.inline_tensor(base_np, name="base_tab")
    identb_d = nc.inline_tensor(np.eye(128, dtype=BF16), name="ident_bf")
    identf_d = nc.inline_tensor(np.eye(128, dtype=np.float32), name="ident_f32")

    with tile.TileContext(nc) as tc:
        with tc.tile_pool(name="const", bufs=1) as cpool:
            xcm_sb = cpool.tile([128, 2 * SLAB_PX], dt.bfloat16)
            wmain_sb = cpool.tile([128, 18 * 256], dt.bfloat16)
            woff_sb = cpool.tile([128, 2 * 9 * 27], dt.bfloat16)
            bias_sb = cpool.tile([27, 1], dt.float32)
            base_sb = cpool.tile([128, 144], dt.float32)
            identb_sb = cpool.tile([128, 128], dt.bfloat16)
            identf_sb = cpool.tile([128, 128], dt.float32)
            nc.sync.dma_start(xcm_sb[:], xcm_d[:])
            nc.sync.dma_start(woff_sb[:], woff_d[:])
            nc.sync.dma_start(bias_sb[:], bias_d[:])
            nc.sync.dma_start(base_sb[:], base_d[:])
            nc.sync.dma_start(identb_sb[:], identb_d[:])
            nc.sync.dma_start(identf_sb[:], identf_d[:])
            nc.sync.dma_start(wmain_sb[:], wmain_d[:])

            with tc.tile_pool(name="wi", bufs=1) as wipool:
                wi_cm = wipool.tile([27, NPX], dt.float32)
                wi_pm = wipool.tile([128, 16 * 27], dt.float32)

                # ---------------- P3: offset conv ----------------
                with tc.tile_pool(name="psA", bufs=2, space="PSUM") as psA:
                    for nt in range(4):
                        ps = psA.tile([27, 512], dt.float32, tag="psA")
                        hh = nt * 8
                        first = True
                        for tap in range(9):
                            ki, kj = tap // 3, tap % 3
                            for ch in range(2):
                                lhsT = woff_sb[:, ch * 243 + tap * 27:
                                               ch * 243 + (tap + 1) * 27]
                                off = ch * SLAB_PX + (hh + ki + 5) * Wp + kj + 5
                                rhs = bass.AP(
                                    xcm_sb.tensor, xcm_sb.offset + off,
                                    [list(xcm_sb.ap[0]), [Wp, 8], [1, 64]])
                                nc.tensor.matmul(
                                    ps[:], lhsT, rhs,
                                    start=first, stop=(tap == 8 and ch == 1))
                                first = False
                        nc.scalar.activation(
                            wi_cm[:, nt * 512:(nt + 1) * 512], ps[:],
                            Act.Identity, bias=bias_sb[:, 0:1], scale=1.0)

                if DEBUG:
                    nc.sync.dma_start(dbg_wicm[:], wi_cm[:])

                # ---------------- PT: transpose wi to pixel-major ----------------
                with tc.tile_pool(name="psB", bufs=3, space="PSUM") as psB:
                    for t in range(16):
                        pst = psB.tile([128, 27], dt.float32, tag="psB")
                        nc.tensor.transpose(
                            pst[:], wi_cm[:, t * 128:(t + 1) * 128],
                            identf_sb[0:27, 0:27])
                        nc.scalar.copy(wi_pm[:, t * 27:(t + 1) * 27], pst[:])

                # ---------------- P4: weights + indices ----------------
                with tc.tile_pool(name="p4", bufs=1) as p4:
                    o1c = p4.tile([128, 144], dt.float32)
                    o2c = p4.tile([128, 144], dt.float32)
                    fo1 = p4.tile([128, 144], dt.float32)
                    fo2 = p4.tile([128, 144], dt.float32)
                    dy = p4.tile([128, 144], dt.float32)
                    dx = p4.tile([128, 144], dt.float32)
                    dy1 = p4.tile([128, 144], dt.float32)
                    dx1 = p4.tile([128, 144], dt.float32)
                    msig = p4.tile([128, 144], dt.float32)
                    w00 = p4.tile([128, 144], dt.float32)
                    w01 = p4.tile([128, 144], dt.float32)
                    w10 = p4.tile([128, 144], dt.float32)
                    w11 = p4.tile([128, 144], dt.float32)
                    ti32 = p4.tile([128, 144], dt.int32)
                    tf32 = p4.tile([128, 144], dt.float32)
                    gcmp = p4.tile([128, 144], dt.float32)
                    idxf = p4.tile([128, 144], dt.float32)
                    idx16 = p4.tile([128, 144], dt.int16)
                    idxw = p4.tile([128, 1152], dt.int16)

                    def wi_view(ch0):
                        return bass.AP(wi_pm.tensor, wi_pm.offset + ch0,
                                       [list(wi_pm.ap[0]), [27, 16], [1, 9]])

                    v = nc.vector
                    v.tensor_scalar(o1c[:], wi_view(0), CLAMP, -CLAMP, Alu.min, Alu.max)
                    v.tensor_scalar(o2c[:], wi_view(9), CLAMP, -CLAMP, Alu.min, Alu.max)
                    nc.scalar.activation(msig[:], wi_view(18), Act.Sigmoid)
                    # floor(o1c) robust to cast rounding mode
                    v.tensor_copy(ti32[:], o1c[:])
                    v.tensor_copy(tf32[:], ti32[:])
                    v.tensor_tensor(gcmp[:], tf32[:], o1c[:], Alu.is_gt)
                    v.tensor_sub(fo1[:], tf32[:], gcmp[:])
                    v.tensor_copy(ti32[:], o2c[:])
                    v.tensor_copy(tf32[:], ti32[:])
                    v.tensor_tensor(gcmp[:], tf32[:], o2c[:], Alu.is_gt)
                    v.tensor_sub(fo2[:], tf32[:], gcmp[:])
                    v.tensor_sub(dy[:], o1c[:], fo1[:])
                    v.tensor_sub(dx[:], o2c[:], fo2[:])
                    v.tensor_scalar(dy1[:], dy[:], -1.0, 1.0, Alu.mult, Alu.add)
                    v.tensor_scalar(dx1[:], dx[:], -1.0, 1.0, Alu.mult, Alu.add)
                    # mask-folded corner weights
                    v.tensor_mul(w00[:], dy1[:], dx1[:])
                    v.tensor_mul(w01[:], dy1[:], dx[:])
                    v.tensor_mul(w10[:], dy[:], dx1[:])
                    v.tensor_mul(w11[:], dy[:], dx[:])
                    v.tensor_mul(w00[:], w00[:], msig[:])
                    v.tensor_mul(w01[:], w01[:], msig[:])
                    v.tensor_mul(w10[:], w10[:], msig[:])
                    v.tensor_mul(w11[:], w11[:], msig[:])
                    # gather record index (top-left corner; record holds y,y+1)
                    v.tensor_scalar_mul(idxf[:], fo1[:], float(Wp))
                    v.tensor_add(idxf[:], idxf[:], fo2[:])
                    v.tensor_add(idxf[:], idxf[:], base_sb[:])
                    # cast to int16 in call-major order: idx16 col = g*36+kk*4+tl
                    pi16 = idx16.ap[0][0]
                    dst = bass.AP(idx16.tensor, idx16.offset,
                                  [[pi16, 128], [36, 4], [1, 4], [4, 9]])
                    v.tensor_copy(dst, bass.AP(
                        idxf.tensor, idxf.offset,
                        [[idxf.ap[0][0], 128], [36, 4], [9, 4], [1, 9]]))

                    # wrap into dma_gather layout: call (g, kk) -> idxw cols
                    # [blk*32, blk*32+32), blk = g*9+kk; slot of local px
                    # l=tl*128+r is (partition r%16, col tl*8 + r//16).
                    for a in range(8):
                        src = bass.AP(idx16.tensor, idx16.offset + 16 * a * pi16,
                                      [[pi16, 16], [1, 144]])
                        dstw = bass.AP(idxw.tensor, idxw.offset + a,
                                       [[idxw.ap[0][0], 16], [32, 36], [8, 4]])
                        nc.sync.dma_start(dstw, src)
                    nc.sync.dma_start(idxw[16:32, :], idxw[0:16, :])
                    nc.sync.dma_start(idxw[32:64, :], idxw[0:32, :])
                    nc.sync.dma_start(idxw[64:128, :], idxw[0:64, :])

                    if DEBUG:
                        nc.sync.dma_start(dbg_wipm[:], wi_pm[:])
                        for i, w_ in enumerate((w00, w01, w10, w11, msig)):
                            nc.sync.dma_start(dbg_w[:, i * 144:(i + 1) * 144], w_[:])
                        nc.sync.dma_start(dbg_idx[:], idx16[:])
                        nc.sync.dma_start(dbg_idxw[:], idxw[:])

                    # ---------------- P5/P6/P7 main loop ----------------
                    # 2-record (2KB) window view of the paired-row slab.
                    gather_src = bass.AP(xrp_d, 0, [[512, NREC], [1, 1024]])
                    # corner slot order within a gathered elem:
                    wmap = (w00, w10, w01, w11)  # offsets 0,256,512,768
                    with tc.tile_pool(name="G", bufs=4) as gpool, \
                         tc.tile_pool(name="samp", bufs=36) as spool, \
                         tc.tile_pool(name="osb", bufs=4) as opool, \
                         tc.tile_pool(name="psC", bufs=4, space="PSUM") as psC, \
                         tc.tile_pool(name="psD", bufs=4, space="PSUM") as psD:
                        for g in range(4):
                            samp = {}
                            for kk in range(KK):
                                gt = gpool.tile([128, 4, 1024], dt.bfloat16, tag="G")
                                blk = (g * 9 + kk) * 32
                                nc.gpsimd.dma_gather(
                                    out_ap=gt[:],
                                    in_ap=gather_src,
                                    idxs_ap=idxw[:, blk:blk + 32],
                                    num_idxs=512,
                                    num_idxs_reg=512,
                                    elem_size=1024,
                                    elem_step=512,
                                )
                                if DEBUG and g == 0:
                                    nc.sync.dma_start(
                                        dbg_g[:, kk * 4096:(kk + 1) * 4096],
                                        bass.AP(gt.tensor, gt.offset,
                                                [list(gt.ap[0]), [1, 4096]]))
                                for tl in range(4):
                                    col = (g * 4 + tl) * 9 + kk
                                    for c4 in range(4):
                                        sl = gt[:, tl, c4 * 256:(c4 + 1) * 256]
                                        if c4 == 3:
                                            nc.scalar.activation(
                                                sl, sl, Act.Copy,
                                                scale=wmap[c4][:, col:col + 1])
                                        else:
                                            v.tensor_scalar_mul(
                                                sl, sl, wmap[c4][:, col:col + 1])
                                for ch in range(2):
                                    ps = psC.tile([128, 512], dt.float32, tag="psC")
                                    for tl in range(4):
                                        for c4 in range(4):
                                            nc.tensor.matmul(
                                                ps[:, tl * 128:(tl + 1) * 128],
                                                gt[:, tl, c4 * 256 + ch * 128:
                                                   c4 * 256 + ch * 128 + 128],
                                                identb_sb[:],
                                                start=(c4 == 0), stop=(c4 == 3))
                                    st = spool.tile([128, 512], dt.bfloat16, tag="samp")
                                    if (kk + ch) % 2 == 0:
                                        v.tensor_copy(st[:], ps[:])
                                    else:
                                        nc.scalar.copy(st[:], ps[:])
                                    samp[(kk, ch)] = st
                                    if DEBUG and g == 0:
                                        bs = (kk * 2 + ch) * 512
                                        nc.sync.dma_start(
                                            dbg_samp[:, bs:bs + 512], st[:])
                            # stage-2
                            for tl in range(4):
                                po = psD.tile([128, 256], dt.float32, tag="psD")
                                n = 0
                                for kk in range(KK):
                                    for ch in range(2):
                                        nc.tensor.matmul(
                                            po[:],
                                            samp[(kk, ch)][:, tl * 128:(tl + 1) * 128],
                                            wmain_sb[:, (kk * 2 + ch) * 256:
                                                     (kk * 2 + ch + 1) * 256],
                                            start=(n == 0), stop=(n == 17))
                                        n += 1
                                ot = opool.tile([128, 256], dt.float32, tag="osb")
                                nc.scalar.copy(ot[:], po[:])
                                row0 = (g * 4 + tl) * 128
                                nc.sync.dma_start(out_d[row0:row0 + 128, :], ot[:])
    nc.finalize()
    return nc


def _host_prep(x, w_offset, b_offset, filt):
    xp = np.zeros((B, 77, Wp, C), dtype=BF16)
    xp[:, PAD:PAD + H, PAD:PAD + W, :] = x.astype(BF16)

    Wm = np.ascontiguousarray(filt.reshape(F, C, KK))
    wmain = np.zeros((128, 18 * 256), dtype=BF16)
    for kk in range(KK):
        for ch in range(2):
            g = kk * 2 + ch
            wmain[:, g * 256:(g + 1) * 256] = Wm[:, ch * 128:(ch + 1) * 128, kk].T.astype(BF16)

    woff = np.zeros((128, 2 * 9 * 27), dtype=BF16)
    for ch in range(2):
        for tap in range(9):
            ki, kj = tap // 3, tap % 3
            woff[:, ch * 243 + tap * 27:ch * 243 + (tap + 1) * 27] = \
                w_offset[ki, kj, ch * 128:(ch + 1) * 128, :].astype(BF16)

    bias = np.ascontiguousarray(b_offset.reshape(27, 1).astype(np.float32))

    in_maps = []
    for core in range(8):
        b, half = core // 2, core % 2
        h0 = 32 * half
        slab = np.ascontiguousarray(xp[b, h0:h0 + SLAB_ROWS].reshape(SLAB_PX, C))
        # paired-row records: rec r = [slab[r], slab[r+76]]
        xrp = np.zeros((NREC + 1, 512), dtype=BF16)
        xrp[:NREC, 0:256] = slab[:NREC]
        xrp[:NREC, 256:512] = slab[Wp:NREC + Wp]
        cm = np.empty((128, 2 * SLAB_PX), dtype=BF16)
        cm[:, 0:SLAB_PX] = slab[:, 0:128].T
        cm[:, SLAB_PX:] = slab[:, 128:256].T
        in_maps.append({
            "xrp": xrp,
            "xslab_cm": np.ascontiguousarray(cm),
            "wmain": wmain,
            "woff": woff,
            "bias": bias,
        })
    return in_maps


def kernel(x, w_offset, b_offset, filt):
    global LAST_RESULT
    x = np.asarray(x, dtype=np.float32)
    w_offset = np.asarray(w_offset, dtype=np.float32)
    b_offset = np.asarray(b_offset, dtype=np.float32)
    filt = np.asarray(filt, dtype=np.float32)

    if "nc" not in _CACHE:
        _CACHE["nc"] = _build_nc()
    nc = _CACHE["nc"]

    from concourse.bass_utils import run_bass_kernel_spmd

    in_maps = _host_prep(x, w_offset, b_offset, filt)
    res = run_bass_kernel_spmd(nc, in_maps, core_ids=list(range(8)))
    LAST_RESULT = res

    out = np.zeros((B, H, W, F), dtype=np.float32)
    for core in range(8):
        b, half = core // 2, core % 2
        out[b, 32 * half:32 * half + 32] = res.results[core]["out"].reshape(32, 64, F)
    return out



# revision 3
# speedup vs baseline: 1.6985x; 1.6985x over previous
"""DeformableConv2D Trainium2 Bass kernel (v2).

Problem: x[4,64,64,256] f32, w_offset[3,3,256,27], b_offset[27], filt[256,256,3,3]
  -> out[4,64,64,256] f32  (3x3 deformable conv, DG=1, SAME padding)

Sharding: 8 cores = (batch b = core//2) x (image-row half = core%2).
Each core computes 32 output rows (2048 pixels) of its batch element.

Key layout trick: the host stages a zero-padded, *paired-row* bf16 copy of
the image: record r=(y,x) holds channels of pixels (y,x) AND (y+1,x).  One
2KB dma_gather descriptor per (tap, pixel) then fetches all 4 bilinear
corners [v00|v10|v01|v11] at once, pixel-major.

Per-core device pipeline (v2):
  P3  offset conv (PE, bf16): wi_cm [27, 2048]
  PT  PE-transpose -> pixel-major wi_pm [128px, 27-per-tile]
  P4  DVE/ACT: clamp, floor, bilinear corner weights (mask-folded),
      int16 gather indices (pixel-major => per-partition scalars)
  IDX partition-fold DMAs (288B runs) + DVE shuffle + replication ladder
      -> idxw [128, 1152] in SWDGE wrapped layout (no 2B-element DMAs)
  WQ  corner weights packed [128, (g,kk)x(tl,cr)] bf16
  DG  per (g,kk): diag tile [128, 16x128] = identity * w (one DVE
      tensor_tensor with stride-0 broadcast APs)
  P5  SWDGE dma_gather (4 queues): per (g, tap) [128px, 4tl, 1024] bf16
  P6  PE matmuls lhsT=gt, rhs=diag(w_corner) accumulating 4 corners in
      PSUM = fused scale + transpose + corner-sum -> sampled [c, px]
      (no per-element scaling on DVE/ACT at all)
  P7  PE matmuls lhsT=sampled[c,px], rhs=W[c,f] -> out [128px, 256f]
"""

import os
import sys
import numpy as np
import ml_dtypes

sys.path.insert(0, "/opt/trn_rl_repo")

BF16 = ml_dtypes.bfloat16

B, H, W, C, F, K, KK = 4, 64, 64, 256, 256, 3, 9
PAD = 6
Wp = 76
SLAB_ROWS = 45           # 44 addressable + 1 zero guard row
SLAB_PX = SLAB_ROWS * Wp  # 3420
NREC = 44 * Wp            # 3344 addressable paired-row records
NPX = 2048
CLAMP = 4.99

_CACHE = {}
LAST_RESULT = None
DEBUG = bool(int(os.environ.get("KERNEL_DEBUG", "0")))


def _build_nc():
    import concourse.bass as bass
    from concourse import bacc, mybir
    import concourse.tile as tile

    dt = mybir.dt
    Alu = mybir.AluOpType
    Act = mybir.ActivationFunctionType

    nc = bacc.Bacc("TRN2", target_bir_lowering=False, num_swdge_queues=4)

    xrp_d = nc.dram_tensor("xrp", [NREC + 1, 512], dt.bfloat16, kind="ExternalInput")
    xcm_d = nc.dram_tensor("xslab_cm", [128, 2 * SLAB_PX], dt.bfloat16, kind="ExternalInput")
    wmain_d = nc.dram_tensor("wmain", [128, 18 * 256], dt.bfloat16, kind="ExternalInput")
    woff_d = nc.dram_tensor("woff", [128, 2 * 9 * 27], dt.bfloat16, kind="ExternalInput")
    bias_d = nc.dram_tensor("bias", [27, 1], dt.float32, kind="ExternalInput")
    out_d = nc.dram_tensor("out", [NPX, C], dt.float32, kind="ExternalOutput")
    if DEBUG:
        dbg_wicm = nc.dram_tensor("dbg_wicm", [27, NPX], dt.float32, kind="ExternalOutput")
        dbg_idxw = nc.dram_tensor("dbg_idxw", [128, 1152], dt.int16, kind="ExternalOutput")
        dbg_wq = nc.dram_tensor("dbg_wq", [128, 576], dt.bfloat16, kind="ExternalOutput")
        dbg_dg = nc.dram_tensor("dbg_dg", [128, 2048], dt.bfloat16, kind="ExternalOutput")
        dbg_samp = nc.dram_tensor("dbg_samp", [128, 18 * 512], dt.bfloat16, kind="ExternalOutput")

    # --- inline constants ---
    base_np = np.zeros((128, 144), dtype=np.float32)
    r = np.arange(128)
    for t in range(16):
        for kk in range(KK):
            ki, kj = kk // 3, kk % 3
            base_np[:, t * 9 + kk] = (2 * t + r // 64 + ki + 5) * Wp + (r % 64) + kj + 5
    base_d = nc.inline_tensor(base_np, name="base_tab")
    identb_d = nc.inline_tensor(np.eye(128, dtype=BF16), name="ident_bf")
    identf_d = nc.inline_tensor(np.eye(128, dtype=np.float32), name="ident_f32")

    with tile.TileContext(nc) as tc:
        with tc.tile_pool(name="const", bufs=1) as cpool:
            xcm_sb = cpool.tile([128, 2 * SLAB_PX], dt.bfloat16)
            wmain_sb = cpool.tile([128, 18 * 256], dt.bfloat16)
            woff_sb = cpool.tile([128, 2 * 9 * 27], dt.bfloat16)
            bias_sb = cpool.tile([27, 1], dt.float32)
            base_sb = cpool.tile([128, 144], dt.float32)
            identb_sb = cpool.tile([128, 128], dt.bfloat16)
            identf_sb = cpool.tile([128, 128], dt.float32)
            nc.sync.dma_start(xcm_sb[:], xcm_d[:])
            nc.sync.dma_start(woff_sb[:], woff_d[:])
            nc.sync.dma_start(bias_sb[:], bias_d[:])
            nc.sync.dma_start(base_sb[:], base_d[:])
            nc.sync.dma_start(identb_sb[:], identb_d[:])
            nc.sync.dma_start(identf_sb[:], identf_d[:])
            nc.sync.dma_start(wmain_sb[:], wmain_d[:])

            with tc.tile_pool(name="wi", bufs=1) as wipool:
                wi_cm = wipool.tile([27, NPX], dt.float32)
                wi_pm = wipool.tile([128, 16 * 27], dt.float32)

                # ---------------- P3: offset conv ----------------
                with tc.tile_pool(name="psA", bufs=2, space="PSUM") as psA:
                    for nt in range(4):
                        ps = psA.tile([27, 512], dt.float32, tag="psA")
                        hh = nt * 8
                        first = True
                        for tap in range(9):
                            ki, kj = tap // 3, tap % 3
                            for ch in range(2):
                                lhsT = woff_sb[:, ch * 243 + tap * 27:
                                               ch * 243 + (tap + 1) * 27]
                                off = ch * SLAB_PX + (hh + ki + 5) * Wp + kj + 5
                                rhs = bass.AP(
                                    xcm_sb.tensor, xcm_sb.offset + off,
                                    [list(xcm_sb.ap[0]), [Wp, 8], [1, 64]])
                                nc.tensor.matmul(
                                    ps[:], lhsT, rhs,
                                    start=first, stop=(tap == 8 and ch == 1))
                                first = False
                        nc.scalar.activation(
                            wi_cm[:, nt * 512:(nt + 1) * 512], ps[:],
                            Act.Identity, bias=bias_sb[:, 0:1], scale=1.0)

                if DEBUG:
                    nc.sync.dma_start(dbg_wicm[:], wi_cm[:])

                # ---------------- PT: transpose wi to pixel-major ----------------
                with tc.tile_pool(name="psB", bufs=3, space="PSUM") as psB:
                    for t in range(16):
                        pst = psB.tile([128, 27], dt.float32, tag="psB")
                        nc.tensor.transpose(
                            pst[:], wi_cm[:, t * 128:(t + 1) * 128],
                            identf_sb[0:27, 0:27])
                        nc.scalar.copy(wi_pm[:, t * 27:(t + 1) * 27], pst[:])

                # ---------------- P4: weights + indices ----------------
                with tc.tile_pool(name="p4", bufs=1) as p4:
                    o1c = p4.tile([128, 144], dt.float32)
                    o2c = p4.tile([128, 144], dt.float32)
                    fo1 = p4.tile([128, 144], dt.float32)
                    fo2 = p4.tile([128, 144], dt.float32)
                    dy = p4.tile([128, 144], dt.float32)
                    dx = p4.tile([128, 144], dt.float32)
                    dy1 = p4.tile([128, 144], dt.float32)
                    dx1 = p4.tile([128, 144], dt.float32)
                    msig = p4.tile([128, 144], dt.float32)
                    w00 = p4.tile([128, 144], dt.float32)
                    w01 = p4.tile([128, 144], dt.float32)
                    w10 = p4.tile([128, 144], dt.float32)
                    w11 = p4.tile([128, 144], dt.float32)
                    ti32 = p4.tile([128, 144], dt.int32)
                    tf32 = p4.tile([128, 144], dt.float32)
                    gcmp = p4.tile([128, 144], dt.float32)
                    idxf = p4.tile([128, 144], dt.float32)
                    idx16 = p4.tile([128, 144], dt.int16)
                    idxq = p4.tile([16, 1152], dt.int16)
                    idxw = p4.tile([128, 1152], dt.int16)
                    wq = p4.tile([128, 576], dt.bfloat16)

                    def wi_view(ch0):
                        return bass.AP(wi_pm.tensor, wi_pm.offset + ch0,
                                       [list(wi_pm.ap[0]), [27, 16], [1, 9]])

                    v = nc.vector
                    v.tensor_scalar(o1c[:], wi_view(0), CLAMP, -CLAMP, Alu.min, Alu.max)
                    v.tensor_scalar(o2c[:], wi_view(9), CLAMP, -CLAMP, Alu.min, Alu.max)
                    nc.scalar.activation(msig[:], wi_view(18), Act.Sigmoid)
                    # floor(o1c) robust to cast rounding mode
                    v.tensor_copy(ti32[:], o1c[:])
                    v.tensor_copy(tf32[:], ti32[:])
                    v.tensor_tensor(gcmp[:], tf32[:], o1c[:], Alu.is_gt)
                    v.tensor_sub(fo1[:], tf32[:], gcmp[:])
                    v.tensor_copy(ti32[:], o2c[:])
                    v.tensor_copy(tf32[:], ti32[:])
                    v.tensor_tensor(gcmp[:], tf32[:], o2c[:], Alu.is_gt)
                    v.tensor_sub(fo2[:], tf32[:], gcmp[:])
                    v.tensor_sub(dy[:], o1c[:], fo1[:])
                    v.tensor_sub(dx[:], o2c[:], fo2[:])
                    v.tensor_scalar(dy1[:], dy[:], -1.0, 1.0, Alu.mult, Alu.add)
                    v.tensor_scalar(dx1[:], dx[:], -1.0, 1.0, Alu.mult, Alu.add)
                    # mask-folded corner weights
                    v.tensor_mul(w00[:], dy1[:], dx1[:])
                    v.tensor_mul(w01[:], dy1[:], dx[:])
                    v.tensor_mul(w10[:], dy[:], dx1[:])
                    v.tensor_mul(w11[:], dy[:], dx[:])
                    v.tensor_mul(w00[:], w00[:], msig[:])
                    v.tensor_mul(w01[:], w01[:], msig[:])
                    v.tensor_mul(w10[:], w10[:], msig[:])
                    v.tensor_mul(w11[:], w11[:], msig[:])
                    # gather record index (top-left corner; record holds y,y+1)
                    v.tensor_scalar_mul(idxf[:], fo1[:], float(Wp))
                    v.tensor_add(idxf[:], idxf[:], fo2[:])
                    v.tensor_add(idxf[:], idxf[:], base_sb[:])
                    # cast to int16 in call-major order: idx16 col = g*36+kk*4+tl
                    pi16 = idx16.ap[0][0]
                    dst = bass.AP(idx16.tensor, idx16.offset,
                                  [[pi16, 128], [36, 4], [1, 4], [4, 9]])
                    v.tensor_copy(dst, bass.AP(
                        idxf.tensor, idxf.offset,
                        [[idxf.ap[0][0], 128], [36, 4], [9, 4], [1, 9]]))

                    # pack corner weights: wq[p, (g*9+kk)*16 + tl*4 + cr]
                    # cr slot order (w00,w10,w01,w11) matches gathered elem
                    # [v00|v10|v01|v11].
                    pw = wq.ap[0][0]
                    for cr, wt in enumerate((w00, w10, w01, w11)):
                        v.tensor_copy(
                            bass.AP(wq.tensor, wq.offset + cr,
                                    [[pw, 128], [144, 4], [16, 9], [4, 4]]),
                            bass.AP(wt.tensor, wt.offset,
                                    [[wt.ap[0][0], 128], [36, 4], [1, 9], [9, 4]]))

                    # ---- idx wrap: fold 128 partitions -> 16, then shuffle ----
                    # idxq[e, q*144 + c] = idx16[q*16+e, c]   (288B runs)
                    piq = idxq.ap[0][0]
                    eng8 = [nc.sync, nc.scalar, nc.gpsimd, nc.sync,
                            nc.scalar, nc.gpsimd, nc.sync, nc.scalar]
                    for q in range(8):
                        eng8[q].dma_start(
                            bass.AP(idxq.tensor, idxq.offset + q * 144,
                                    [[piq, 16], [1, 144]]),
                            bass.AP(idx16.tensor, idx16.offset + 16 * q * pi16,
                                    [[pi16, 16], [1, 144]]))
                    # idxw[e, g*288 + (kk*4+tl)*8 + q] = idxq[e, q*144 + g*36 + kk*4+tl]
                    piw = idxw.ap[0][0]
                    v.tensor_copy(
                        bass.AP(idxw.tensor, idxw.offset,
                                [[piw, 16], [288, 4], [8, 36], [1, 8]]),
                        bass.AP(idxq.tensor, idxq.offset,
                                [[piq, 16], [36, 4], [1, 36], [144, 8]]))
                    # replicate to all 128 partitions (8 gpsimd cores)
                    nc.sync.dma_start(idxw[16:32, :], idxw[0:16, :])
                    nc.sync.dma_start(idxw[32:64, :], idxw[0:32, :])
                    nc.sync.dma_start(idxw[64:128, :], idxw[0:64, :])

                    if DEBUG:
                        nc.sync.dma_start(dbg_idxw[:], idxw[:])
                        nc.sync.dma_start(dbg_wq[:], wq[:])

                    # ---------------- P5/P6/P7 main loop ----------------
                    # 2-record (2KB) window view of the paired-row slab.
                    gather_src = bass.AP(xrp_d, 0, [[512, NREC], [1, 1024]])
                    pid = identb_sb.ap[0][0]
                    with tc.tile_pool(name="G", bufs=5) as gpool, \
                         tc.tile_pool(name="DG", bufs=6) as dgpool, \
                         tc.tile_pool(name="samp", bufs=36) as spool, \
                         tc.tile_pool(name="osb", bufs=4) as opool, \
                         tc.tile_pool(name="psC", bufs=4, space="PSUM") as psC, \
                         tc.tile_pool(name="psD", bufs=4, space="PSUM") as psD:
                        for g in range(4):
                            samp = {}
                            for kk in range(KK):
                                blk = g * 9 + kk
                                # diag tile: dg[:, (tl*4+cr)*128 + j] =
                                #   I[p, j] * wq[p, blk*16 + tl*4 + cr]
                                dg = dgpool.tile([128, 2048], dt.bfloat16, tag="DG")
                                v.tensor_tensor(
                                    bass.AP(dg.tensor, dg.offset,
                                            [[dg.ap[0][0], 128], [128, 16], [1, 128]]),
                                    bass.AP(identb_sb.tensor, identb_sb.offset,
                                            [[pid, 128], [0, 16], [1, 128]]),
                                    bass.AP(wq.tensor, wq.offset + blk * 16,
                                            [[pw, 128], [1, 16], [0, 128]]),
                                    Alu.mult)
                                gt = gpool.tile([128, 4, 1024], dt.bfloat16, tag="G")
                                nc.gpsimd.dma_gather(
                                    out_ap=gt[:],
                                    in_ap=gather_src,
                                    idxs_ap=idxw[:, blk * 32:blk * 32 + 32],
                                    num_idxs=512,
                                    num_idxs_reg=512,
                                    elem_size=1024,
                                    elem_step=512,
                                    queue_num=blk % 4,
                                )
                                if DEBUG and g == 0 and kk == 0:
                                    nc.sync.dma_start(dbg_dg[:], dg[:])
                                for ch in range(2):
                                    ps = psC.tile([128, 512], dt.float32, tag="psC")
                                    for tl in range(4):
                                        for cr in range(4):
                                            nc.tensor.matmul(
                                                ps[:, tl * 128:(tl + 1) * 128],
                                                gt[:, tl, cr * 256 + ch * 128:
                                                   cr * 256 + ch * 128 + 128],
                                                dg[:, (tl * 4 + cr) * 128:
                                                   (tl * 4 + cr + 1) * 128],
                                                start=(cr == 0), stop=(cr == 3))
                                    st = spool.tile([128, 512], dt.bfloat16, tag="samp")
                                    if (kk + ch) % 2 == 0:
                                        v.tensor_copy(st[:], ps[:])
                                    else:
                                        nc.scalar.copy(st[:], ps[:])
                                    samp[(kk, ch)] = st
                                    if DEBUG and g == 0:
                                        bs = (kk * 2 + ch) * 512
                                        nc.sync.dma_start(
                                            dbg_samp[:, bs:bs + 512], st[:])
                            # stage-2
                            for tl in range(4):
                                po = psD.tile([128, 256], dt.float32, tag="psD")
                                n = 0
                                for kk in range(KK):
                                    for ch in range(2):
                                        nc.tensor.matmul(
                                            po[:],
                                            samp[(kk, ch)][:, tl * 128:(tl + 1) * 128],
                                            wmain_sb[:, (kk * 2 + ch) * 256:
                                                     (kk * 2 + ch + 1) * 256],
                                            start=(n == 0), stop=(n == 17))
                                        n += 1
                                ot = opool.tile([128, 256], dt.float32, tag="osb")
                                nc.scalar.copy(ot[:], po[:])
                                row0 = (g * 4 + tl) * 128
                                nc.sync.dma_start(out_d[row0:row0 + 128, :], ot[:])
    nc.finalize()
    return nc


def _host_prep(x, w_offset, b_offset, filt):
    xp = np.zeros((B, 77, Wp, C), dtype=BF16)
    xp[:, PAD:PAD + H, PAD:PAD + W, :] = x.astype(BF16)

    Wm = np.ascontiguousarray(filt.reshape(F, C, KK))
    wmain = np.zeros((128, 18 * 256), dtype=BF16)
    for kk in range(KK):
        for ch in range(2):
            g = kk * 2 + ch
            wmain[:, g * 256:(g + 1) * 256] = Wm[:, ch * 128:(ch + 1) * 128, kk].T.astype(BF16)

    woff = np.zeros((128, 2 * 9 * 27), dtype=BF16)
    for ch in range(2):
        for tap in range(9):
            ki, kj = tap // 3, tap % 3
            woff[:, ch * 243 + tap * 27:ch * 243 + (tap + 1) * 27] = \
                w_offset[ki, kj, ch * 128:(ch + 1) * 128, :].astype(BF16)

    bias = np.ascontiguousarray(b_offset.reshape(27, 1).astype(np.float32))

    in_maps = []
    for core in range(8):
        b, half = core // 2, core % 2
        h0 = 32 * half
        slab = np.ascontiguousarray(xp[b, h0:h0 + SLAB_ROWS].reshape(SLAB_PX, C))
        # paired-row records: rec r = [slab[r], slab[r+76]]
        xrp = np.zeros((NREC + 1, 512), dtype=BF16)
        xrp[:NREC, 0:256] = slab[:NREC]
        xrp[:NREC, 256:512] = slab[Wp:NREC + Wp]
        cm = np.empty((128, 2 * SLAB_PX), dtype=BF16)
        cm[:, 0:SLAB_PX] = slab[:, 0:128].T
        cm[:, SLAB_PX:] = slab[:, 128:256].T
        in_maps.append({
            "xrp": xrp,
            "xslab_cm": np.ascontiguousarray(cm),
            "wmain": wmain,
            "woff": woff,
            "bias": bias,
        })
    return in_maps


def kernel(x, w_offset, b_offset, filt):
    global LAST_RESULT
    x = np.asarray(x, dtype=np.float32)
    w_offset = np.asarray(w_offset, dtype=np.float32)
    b_offset = np.asarray(b_offset, dtype=np.float32)
    filt = np.asarray(filt, dtype=np.float32)

    if "nc" not in _CACHE:
        _CACHE["nc"] = _build_nc()
    nc = _CACHE["nc"]

    from concourse.bass_utils import run_bass_kernel_spmd

    in_maps = _host_prep(x, w_offset, b_offset, filt)
    res = run_bass_kernel_spmd(nc, in_maps, core_ids=list(range(8)))
    LAST_RESULT = res

    out = np.zeros((B, H, W, F), dtype=np.float32)
    for core in range(8):
        b, half = core // 2, core % 2
        out[b, 32 * half:32 * half + 32] = res.results[core]["out"].reshape(32, 64, F)
    return out


# revision 8
# speedup vs baseline: 1.7621x; 1.0375x over previous
"""DeformableConv2D Trainium2 Bass kernel (v3).

Problem: x[4,64,64,256] f32, w_offset[3,3,256,27], b_offset[27], filt[256,256,3,3]
  -> out[4,64,64,256] f32  (3x3 deformable conv, DG=1, SAME padding)

Sharding: 8 cores = (batch b = core//2) x (image-row half = core%2).
Each core computes 32 output rows (2048 pixels) of its batch element.

Key layout trick: the host stages a zero-padded, *paired-row* bf16 copy of
the image: record r=(y,x) holds channels of pixels (y,x) AND (y+1,x).  One
2KB dma_gather descriptor per (tap, pixel) then fetches all 4 bilinear
corners [v00|v10|v01|v11] at once, pixel-major.

v3 structure (front-end split so gathers start ~30us in):
  split s=0 covers pixel group g=0, split s=1 covers g=1..3.
  P3  offset conv per row band (PE, bf16): wi_cm [27, 2048]
  PT  PE-transpose -> pixel-major wi_pm
  P4  per-split DVE: clamp, floor, corner weights, gather indices
  IDX per-split: partition-fold DMAs (288B runs) + DVE shuffle +
      parallel replication ladder -> idxw [128, 1152]
      split-0 DMAs on gpsimd+sync (early), split-1 on sync only so they
      never block gathers or sampled copies.
  DG  per (g,kk): diag tile [128, 16x128] = identity * w (stride-0 TT),
      all on DVE, pre-built ahead of consumption
  P5  SWDGE dma_gather on 4 queues
  P6  PE matmuls lhsT=gt, rhs=diag(w) accumulating 4 corners in PSUM
      (fused scale+transpose+corner-sum) -> sampled [c, px]
  P7  incremental stage-2: po[tl] += sampled.T @ W right after each
      (kk,ch) sampled tile lands (no end-of-group tail); PSUM->SBUF
      copies all on the scalar engine.
"""

import os
import sys
import numpy as np
import ml_dtypes

sys.path.insert(0, "/opt/trn_rl_repo")

BF16 = ml_dtypes.bfloat16

B, H, W, C, F, K, KK = 4, 64, 64, 256, 256, 3, 9
PAD = 6
Wp = 76
SLAB_ROWS = 45           # 44 addressable + 1 zero guard row
SLAB_PX = SLAB_ROWS * Wp  # 3420
NREC = 44 * Wp            # 3344 addressable paired-row records
NPX = 2048
CLAMP = 4.99

_CACHE = {}
LAST_RESULT = None
DEBUG = bool(int(os.environ.get("KERNEL_DEBUG", "0")))


def _build_nc():
    import concourse.bass as bass
    from concourse import bacc, mybir
    import concourse.tile as tile

    dt = mybir.dt
    Alu = mybir.AluOpType
    Act = mybir.ActivationFunctionType

    nc = bacc.Bacc("TRN2", target_bir_lowering=False, num_swdge_queues=4)

    xrp_d = nc.dram_tensor("xrp", [NREC + 1, 512], dt.bfloat16, kind="ExternalInput")
    xcm_d = nc.dram_tensor("xslab_cm", [128, 2 * SLAB_PX], dt.bfloat16, kind="ExternalInput")
    wmain_d = nc.dram_tensor("wmain", [128, 18 * 256], dt.bfloat16, kind="ExternalInput")
    woff_d = nc.dram_tensor("woff", [128, 2 * 9 * 27], dt.bfloat16, kind="ExternalInput")
    bias_d = nc.dram_tensor("bias", [27, 1], dt.float32, kind="ExternalInput")
    out_d = nc.dram_tensor("out", [NPX, C], dt.float32, kind="ExternalOutput")
    if DEBUG:
        dbg_wicm = nc.dram_tensor("dbg_wicm", [27, NPX], dt.float32, kind="ExternalOutput")
        dbg_idxw = nc.dram_tensor("dbg_idxw", [128, 1152], dt.int16, kind="ExternalOutput")
        dbg_wq = nc.dram_tensor("dbg_wq", [128, 576], dt.bfloat16, kind="ExternalOutput")
        dbg_samp = nc.dram_tensor("dbg_samp", [128, 18 * 512], dt.bfloat16, kind="ExternalOutput")

    # --- inline constants ---
    base_np = np.zeros((128, 144), dtype=np.float32)
    r = np.arange(128)
    for t in range(16):
        for kk in range(KK):
            ki, kj = kk // 3, kk % 3
            base_np[:, t * 9 + kk] = (2 * t + r // 64 + ki + 5) * Wp + (r % 64) + kj + 5
    base_d = nc.inline_tensor(base_np, name="base_tab")
    identb_d = nc.inline_tensor(np.eye(128, dtype=BF16), name="ident_bf")
    identf_d = nc.inline_tensor(np.eye(128, dtype=np.float32), name="ident_f32")

    with tile.TileContext(nc) as tc:
        with tc.tile_pool(name="const", bufs=1) as cpool:
            xcm_sb = cpool.tile([128, 2 * SLAB_PX], dt.bfloat16)
            wmain_sb = cpool.tile([128, 18 * 256], dt.bfloat16)
            woff_sb = cpool.tile([128, 2 * 9 * 27], dt.bfloat16)
            bias_sb = cpool.tile([27, 1], dt.float32)
            base_sb = cpool.tile([128, 144], dt.float32)
            identb_sb = cpool.tile([128, 128], dt.bfloat16)
            identf_sb = cpool.tile([128, 128], dt.float32)
            nc.sync.dma_start(woff_sb[:], woff_d[:])
            nc.sync.dma_start(bias_sb[:], bias_d[:])
            nc.scalar.dma_start(base_sb[:], base_d[:])
            nc.scalar.dma_start(identb_sb[:], identb_d[:])
            nc.scalar.dma_start(identf_sb[:], identf_d[:])
            # xcm split: rows 0-15 first so P3(nt0) can start early
            nc.sync.dma_start(
                bass.AP(xcm_sb.tensor, xcm_sb.offset,
                        [list(xcm_sb.ap[0]), [SLAB_PX, 2], [1, 16 * Wp]]),
                bass.AP(xcm_d, 0, [[2 * SLAB_PX, 128], [SLAB_PX, 2], [1, 16 * Wp]]))
            nc.sync.dma_start(
                bass.AP(xcm_sb.tensor, xcm_sb.offset + 16 * Wp,
                        [list(xcm_sb.ap[0]), [SLAB_PX, 2], [1, SLAB_PX - 16 * Wp]]),
                bass.AP(xcm_d, 16 * Wp,
                        [[2 * SLAB_PX, 128], [SLAB_PX, 2], [1, SLAB_PX - 16 * Wp]]))
            nc.gpsimd.dma_start(wmain_sb[:], wmain_d[:])

            with tc.tile_pool(name="wi", bufs=1) as wipool:
                wi_cm = wipool.tile([27, NPX], dt.float32)
                wi_pm = wipool.tile([128, 16 * 27], dt.float32)

                with tc.tile_pool(name="p4", bufs=1) as p4:
                    o1c = p4.tile([128, 144], dt.float32)
                    o2c = p4.tile([128, 144], dt.float32)
                    fo1 = p4.tile([128, 144], dt.float32)
                    fo2 = p4.tile([128, 144], dt.float32)
                    dy = p4.tile([128, 144], dt.float32)
                    dx = p4.tile([128, 144], dt.float32)
                    dy1 = p4.tile([128, 144], dt.float32)
                    dx1 = p4.tile([128, 144], dt.float32)
                    msig = p4.tile([128, 144], dt.float32)
                    w00 = p4.tile([128, 144], dt.float32)
                    w01 = p4.tile([128, 144], dt.float32)
                    w10 = p4.tile([128, 144], dt.float32)
                    w11 = p4.tile([128, 144], dt.float32)
                    ti32 = p4.tile([128, 144], dt.int32)
                    tf32 = p4.tile([128, 144], dt.float32)
                    gcmp = p4.tile([128, 144], dt.float32)
                    idxf = p4.tile([128, 144], dt.float32)
                    idx16 = p4.tile([128, 144], dt.int16)
                    idxq = p4.tile([16, 1152], dt.int16)
                    idxw = p4.tile([128, 1152], dt.int16)
                    wq = p4.tile([128, 576], dt.bfloat16)

                    v = nc.vector
                    pi16 = idx16.ap[0][0]
                    piq = idxq.ap[0][0]
                    piw = idxw.ap[0][0]
                    pw = wq.ap[0][0]
                    pid = identb_sb.ap[0][0]

                    def p3_band(nt, psA):
                        ps = psA.tile([27, 512], dt.float32, tag="psA")
                        hh = nt * 8
                        first = True
                        for tap in range(9):
                            ki, kj = tap // 3, tap % 3
                            for ch in range(2):
                                lhsT = woff_sb[:, ch * 243 + tap * 27:
                                               ch * 243 + (tap + 1) * 27]
                                off = ch * SLAB_PX + (hh + ki + 5) * Wp + kj + 5
                                rhs = bass.AP(
                                    xcm_sb.tensor, xcm_sb.offset + off,
                                    [list(xcm_sb.ap[0]), [Wp, 8], [1, 64]])
                                nc.tensor.matmul(
                                    ps[:], lhsT, rhs,
                                    start=first, stop=(tap == 8 and ch == 1))
                                first = False
                        nc.scalar.activation(
                            wi_cm[:, nt * 512:(nt + 1) * 512], ps[:],
                            Act.Identity, bias=bias_sb[:, 0:1], scale=1.0)

                    def pt_band(nt, psB):
                        for t in range(4 * nt, 4 * nt + 4):
                            pst = psB.tile([128, 27], dt.float32, tag="psB")
                            nc.tensor.transpose(
                                pst[:], wi_cm[:, t * 128:(t + 1) * 128],
                                identf_sb[0:27, 0:27])
                            nc.scalar.copy(wi_pm[:, t * 27:(t + 1) * 27], pst[:])

                    def wi_view(ch0, c0, nc_):
                        return bass.AP(wi_pm.tensor, wi_pm.offset + ch0 + 27 * c0,
                                       [list(wi_pm.ap[0]), [27, nc_], [1, 9]])

                    def sl(t_, c0, nc_):
                        return t_[:, 9 * c0:9 * (c0 + nc_)]

                    def p4_split(c0, nc_):
                        # per-split P4 math on tile-columns [c0, c0+nc_)
                        v.tensor_scalar(sl(o1c, c0, nc_), wi_view(0, c0, nc_),
                                        CLAMP, -CLAMP, Alu.min, Alu.max)
                        v.tensor_scalar(sl(o2c, c0, nc_), wi_view(9, c0, nc_),
                                        CLAMP, -CLAMP, Alu.min, Alu.max)
                        nc.scalar.activation(sl(msig, c0, nc_),
                                             wi_view(18, c0, nc_), Act.Sigmoid)
                        v.tensor_copy(sl(ti32, c0, nc_), sl(o1c, c0, nc_))
                        v.tensor_copy(sl(tf32, c0, nc_), sl(ti32, c0, nc_))
                        v.tensor_tensor(sl(gcmp, c0, nc_), sl(tf32, c0, nc_),
                                        sl(o1c, c0, nc_), Alu.is_gt)
                        v.tensor_sub(sl(fo1, c0, nc_), sl(tf32, c0, nc_),
                                     sl(gcmp, c0, nc_))
                        v.tensor_copy(sl(ti32, c0, nc_), sl(o2c, c0, nc_))
                        v.tensor_copy(sl(tf32, c0, nc_), sl(ti32, c0, nc_))
                        v.tensor_tensor(sl(gcmp, c0, nc_), sl(tf32, c0, nc_),
                                        sl(o2c, c0, nc_), Alu.is_gt)
                        v.tensor_sub(sl(fo2, c0, nc_), sl(tf32, c0, nc_),
                                     sl(gcmp, c0, nc_))
                        v.tensor_sub(sl(dy, c0, nc_), sl(o1c, c0, nc_),
                                     sl(fo1, c0, nc_))
                        v.tensor_sub(sl(dx, c0, nc_), sl(o2c, c0, nc_),
                                     sl(fo2, c0, nc_))
                        v.tensor_scalar(sl(dy1, c0, nc_), sl(dy, c0, nc_),
                                        -1.0, 1.0, Alu.mult, Alu.add)
                        v.tensor_scalar(sl(dx1, c0, nc_), sl(dx, c0, nc_),
                                        -1.0, 1.0, Alu.mult, Alu.add)
                        v.tensor_mul(sl(w00, c0, nc_), sl(dy1, c0, nc_), sl(dx1, c0, nc_))
                        v.tensor_mul(sl(w01, c0, nc_), sl(dy1, c0, nc_), sl(dx, c0, nc_))
                        v.tensor_mul(sl(w10, c0, nc_), sl(dy, c0, nc_), sl(dx1, c0, nc_))
                        v.tensor_mul(sl(w11, c0, nc_), sl(dy, c0, nc_), sl(dx, c0, nc_))
                        v.tensor_mul(sl(w00, c0, nc_), sl(w00, c0, nc_), sl(msig, c0, nc_))
                        v.tensor_mul(sl(w01, c0, nc_), sl(w01, c0, nc_), sl(msig, c0, nc_))
                        v.tensor_mul(sl(w10, c0, nc_), sl(w10, c0, nc_), sl(msig, c0, nc_))
                        v.tensor_mul(sl(w11, c0, nc_), sl(w11, c0, nc_), sl(msig, c0, nc_))
                        v.tensor_scalar_mul(sl(idxf, c0, nc_), sl(fo1, c0, nc_), float(Wp))
                        v.tensor_add(sl(idxf, c0, nc_), sl(idxf, c0, nc_),
                                     sl(fo2, c0, nc_))
                        v.tensor_add(sl(idxf, c0, nc_), sl(idxf, c0, nc_),
                                     bass.AP(base_sb.tensor, base_sb.offset + 9 * c0,
                                             [list(base_sb.ap[0]), [1, 9 * nc_]]))
                        # int16 cast in call-major order: idx16 col = g*36+kk*4+tl
                        ng = nc_ // 4
                        v.tensor_copy(
                            bass.AP(idx16.tensor, idx16.offset + 9 * c0,
                                    [[pi16, 128], [36, ng], [1, 4], [4, 9]]),
                            bass.AP(idxf.tensor, idxf.offset + 9 * c0,
                                    [[idxf.ap[0][0], 128], [36, ng], [9, 4], [1, 9]]))

                    def wq_split(c0, nc_):
                        ng = nc_ // 4
                        for cr, wt in enumerate((w00, w10, w01, w11)):
                            v.tensor_copy(
                                bass.AP(wq.tensor, wq.offset + 36 * c0 + cr,
                                        [[pw, 128], [144, ng], [16, 9], [4, 4]]),
                                bass.AP(wt.tensor, wt.offset + 9 * c0,
                                        [[wt.ap[0][0], 128], [36, ng], [1, 9], [9, 4]]))

                    def idx_fold(c0, nc_, engs):
                        # fold 128 partitions -> 16 (contiguous runs)
                        w9 = 9 * nc_
                        for q in range(8):
                            engs[q % len(engs)].dma_start(
                                bass.AP(idxq.tensor, idxq.offset + 72 * c0 + q * w9,
                                        [[piq, 16], [1, w9]]),
                                bass.AP(idx16.tensor,
                                        idx16.offset + 16 * q * pi16 + 9 * c0,
                                        [[pi16, 16], [1, w9]]))

                    def idx_shuffle(c0, nc_):
                        # idxw[e, g*288+(kk*4+tl)*8+q] =
                        #   idxq[e, 72*c0 + q*9*nc_ + g*36 + kk*4+tl]
                        ng = nc_ // 4
                        w9 = 9 * nc_
                        v.tensor_copy(
                            bass.AP(idxw.tensor, idxw.offset + 72 * c0,
                                    [[piw, 16], [288, ng], [8, 36], [1, 8]]),
                            bass.AP(idxq.tensor, idxq.offset + 72 * c0,
                                    [[piq, 16], [36, ng], [1, 36], [w9, 8]]))

                    def idx_ladder(c0, nc_, engs):
                        for j in range(1, 8):
                            engs[(j - 1) % len(engs)].dma_start(
                                idxw[16 * j:16 * (j + 1),
                                     72 * c0:72 * (c0 + nc_)],
                                idxw[0:16, 72 * c0:72 * (c0 + nc_)])

                    def dg_build(g, kk, dgpool):
                        blk = g * 9 + kk
                        dg = dgpool.tile([128, 2048], dt.bfloat16, tag="DG",
                                         name=f"dg_{g}_{kk}")
                        v.tensor_tensor(
                            bass.AP(dg.tensor, dg.offset,
                                    [[dg.ap[0][0], 128], [128, 16], [1, 128]]),
                            bass.AP(identb_sb.tensor, identb_sb.offset,
                                    [[pid, 128], [0, 16], [1, 128]]),
                            bass.AP(wq.tensor, wq.offset + blk * 16,
                                    [[pw, 128], [1, 16], [0, 128]]),
                            Alu.mult)
                        return dg

                    # ---- front-end, split-pipelined ----
                    dgt = {}
                    with tc.tile_pool(name="DG", bufs=12) as dgpool:
                        with tc.tile_pool(name="psA", bufs=2, space="PSUM") as psA, \
                             tc.tile_pool(name="psB", bufs=3, space="PSUM") as psB:
                            p3_band(0, psA)
                            pt_band(0, psB)
                            for nt in (1, 2, 3):
                                p3_band(nt, psA)
                                pt_band(nt, psB)
                            p4_split(0, 4)       # g=0
                            idx_fold(0, 4, [nc.gpsimd, nc.sync])
                            idx_shuffle(0, 4)
                            idx_ladder(0, 4, [nc.gpsimd, nc.sync])
                            wq_split(0, 4)
                            for kk in range(3):
                                dgt[(0, kk)] = dg_build(0, kk, dgpool)
                            p4_split(4, 12)      # g=1..3
                            idx_fold(4, 12, [nc.sync])
                            for kk in range(3, 9):
                                dgt[(0, kk)] = dg_build(0, kk, dgpool)
                            idx_shuffle(4, 12)
                            idx_ladder(4, 12, [nc.sync])
                            wq_split(4, 12)

                        if DEBUG:
                            nc.sync.dma_start(dbg_wicm[:], wi_cm[:])
                            nc.sync.dma_start(dbg_idxw[:], idxw[:])
                            nc.sync.dma_start(dbg_wq[:], wq[:])

                        # ---------------- P5/P6/P7 main loop ----------------
                        gather_src = bass.AP(xrp_d, 0, [[512, NREC], [1, 1024]])
                        with tc.tile_pool(name="G", bufs=5) as gpool, \
                             tc.tile_pool(name="samp", bufs=8) as spool, \
                             tc.tile_pool(name="osb", bufs=4) as opool, \
                             tc.tile_pool(name="psC", bufs=3, space="PSUM") as psC, \
                             tc.tile_pool(name="psD", bufs=4, space="PSUM") as psD:
                            for g in range(4):
                                po = [psD.tile([128, 256], dt.float32, tag="psD",
                                               name=f"po_{g}_{tl}")
                                      for tl in range(4)]
                                for kk in range(KK):
                                    blk = g * 9 + kk
                                    dg = dgt.pop((g, kk), None)
                                    if dg is None:
                                        dg = dg_build(g, kk, dgpool)
                                    gt = gpool.tile([128, 4, 1024], dt.bfloat16, tag="G")
                                    nc.gpsimd.dma_gather(
                                        out_ap=gt[:],
                                        in_ap=gather_src,
                                        idxs_ap=idxw[:, blk * 32:blk * 32 + 32],
                                        num_idxs=512,
                                        num_idxs_reg=512,
                                        elem_size=1024,
                                        elem_step=512,
                                        queue_num=blk % 4,
                                    )
                                    for ch in range(2):
                                        ps = psC.tile([128, 512], dt.float32, tag="psC")
                                        for tl in range(4):
                                            for cr in range(4):
                                                nc.tensor.matmul(
                                                    ps[:, tl * 128:(tl + 1) * 128],
                                                    gt[:, tl, cr * 256 + ch * 128:
                                                       cr * 256 + ch * 128 + 128],
                                                    dg[:, (tl * 4 + cr) * 128:
                                                       (tl * 4 + cr + 1) * 128],
                                                    start=(cr == 0), stop=(cr == 3))
                                        st = spool.tile([128, 512], dt.bfloat16,
                                                        tag="samp")
                                        nc.scalar.copy(st[:], ps[:])
                                        if DEBUG and g == 0:
                                            bs = (kk * 2 + ch) * 512
                                            nc.sync.dma_start(
                                                dbg_samp[:, bs:bs + 512], st[:])
                                        # incremental stage-2
                                        n = kk * 2 + ch
                                        for tl in range(4):
                                            nc.tensor.matmul(
                                                po[tl][:],
                                                st[:, tl * 128:(tl + 1) * 128],
                                                wmain_sb[:, n * 256:(n + 1) * 256],
                                                start=(n == 0), stop=(n == 17))
                                for tl in range(4):
                                    ot = opool.tile([128, 256], dt.float32, tag="osb")
                                    nc.scalar.copy(ot[:], po[tl][:])
                                    row0 = (g * 4 + tl) * 128
                                    nc.sync.dma_start(out_d[row0:row0 + 128, :], ot[:])
    nc.finalize()
    return nc


def _host_prep(x, w_offset, b_offset, filt):
    xp = np.zeros((B, 77, Wp, C), dtype=BF16)
    xp[:, PAD:PAD + H, PAD:PAD + W, :] = x.astype(BF16)

    Wm = np.ascontiguousarray(filt.reshape(F, C, KK))
    wmain = np.zeros((128, 18 * 256), dtype=BF16)
    for kk in range(KK):
        for ch in range(2):
            g = kk * 2 + ch
            wmain[:, g * 256:(g + 1) * 256] = Wm[:, ch * 128:(ch + 1) * 128, kk].T.astype(BF16)

    woff = np.zeros((128, 2 * 9 * 27), dtype=BF16)
    for ch in range(2):
        for tap in range(9):
            ki, kj = tap // 3, tap % 3
            woff[:, ch * 243 + tap * 27:ch * 243 + (tap + 1) * 27] = \
                w_offset[ki, kj, ch * 128:(ch + 1) * 128, :].astype(BF16)

    bias = np.ascontiguousarray(b_offset.reshape(27, 1).astype(np.float32))

    in_maps = []
    for core in range(8):
        b, half = core // 2, core % 2
        h0 = 32 * half
        slab = np.ascontiguousarray(xp[b, h0:h0 + SLAB_ROWS].reshape(SLAB_PX, C))
        # paired-row records: rec r = [slab[r], slab[r+76]]
        xrp = np.zeros((NREC + 1, 512), dtype=BF16)
        xrp[:NREC, 0:256] = slab[:NREC]
        xrp[:NREC, 256:512] = slab[Wp:NREC + Wp]
        cm = np.empty((128, 2 * SLAB_PX), dtype=BF16)
        cm[:, 0:SLAB_PX] = slab[:, 0:128].T
        cm[:, SLAB_PX:] = slab[:, 128:256].T
        in_maps.append({
            "xrp": xrp,
            "xslab_cm": np.ascontiguousarray(cm),
            "wmain": wmain,
            "woff": woff,
            "bias": bias,
        })
    return in_maps


def kernel(x, w_offset, b_offset, filt):
    global LAST_RESULT
    x = np.asarray(x, dtype=np.float32)
    w_offset = np.asarray(w_offset, dtype=np.float32)
    b_offset = np.asarray(b_offset, dtype=np.float32)
    filt = np.asarray(filt, dtype=np.float32)

    if "nc" not in _CACHE:
        _CACHE["nc"] = _build_nc()
    nc = _CACHE["nc"]

    from concourse.bass_utils import run_bass_kernel_spmd

    in_maps = _host_prep(x, w_offset, b_offset, filt)
    res = run_bass_kernel_spmd(nc, in_maps, core_ids=list(range(8)))
    LAST_RESULT = res

    out = np.zeros((B, H, W, F), dtype=np.float32)
    for core in range(8):
        b, half = core // 2, core % 2
        out[b, 32 * half:32 * half + 32] = res.results[core]["out"].reshape(32, 64, F)
    return out
